# revision 70
# baseline (speedup 1.0000x reference)
"""GCN layer (GCNConv + log_softmax) on 8 Trainium2 NeuronCores.

Sharding: nodes row-sharded 8 ways. Each core computes h' = dis_src * (x @ W)
for its slice in bf16, casts to fp8e4, AllGathers h' in two chunks (A: first
3584 local rows, gathered early; B: last 2688), then aggregates messages for
its destination slice with dst-tile dma_gathers (descriptors pre-generated on
the Pool engine via prepare_only, triggered when the AllGathers land and the
target SBUF slot frees) + host-precomputed 0/1 indicator matmuls in fp8
DoubleRow mode on the tensor engine. Epilogue applies dis_dst (scalar-engine
scale), bias, and log_softmax without max-subtraction (|z| is small), with Ln
batched per tile-chunk; output written fp16 and widened to f32 on host.
"""

import numpy as np
import ml_dtypes

import concourse.bass as bass
import concourse.tile as tile
from concourse import bacc, mybir
from concourse.bass import _add_dep_helper
from concourse.bass_utils import run_bass_kernel_spmd

bf16 = ml_dtypes.bfloat16
fp8 = ml_dtypes.float8_e4m3
F32 = mybir.dt.float32
F16 = mybir.dt.float16
BF16 = mybir.dt.bfloat16
FP8 = mybir.dt.float8e4
I16 = mybir.dt.int16

N_NODES = 50000
D_IN = 2048
D_OUT = 512
C = 8                      # cores
NLOC = N_NODES // C        # 6250 real nodes per core
T = 49                     # dst tiles per core
NPAD = T * 128             # 6272 padded rows per core
SUP = 7                    # GEMM row-chunks per core
SW = NPAD // SUP           # 896 rows per chunk
SUP_A = 3                  # chunks feeding AllGather A (fires early)
LOC_A = SUP_A * SW         # 3584 local rows in A
LOC_B = NPAD - LOC_A       # 2688 local rows in B
ROWS_A = C * LOC_A         # 28672 rows in h_A  (< 32768 -> int16 ok)
ROWS_B = C * LOC_B         # 21504 rows in h_B
KT = D_IN // 128           # 16 contraction chunks
NBUF = 8                   # rotating gather slots per half
LCH = 7                    # tiles per Ln batch
PREP_MODE = False          # prepare_only/trigger pipeline deadlocks Tile's
                           # SWDGE lane accounting — keep immediate gathers

LAST_RESULTS = None        # test harness reads exec_time_ns from here

DR = mybir.MatmulPerfMode.DoubleRow


def _wrap_idx(idx):
    """Wrap a [n] index array into the [128, n//16] dma_gather layout."""
    n = idx.shape[0]
    assert n % 16 == 0
    cols = n // 16
    w = np.empty((128, cols), np.int16)
    blk = idx.reshape(cols, 16).T.astype(np.int16)   # [16, cols]
    for g in range(8):
        w[g * 16:(g + 1) * 16, :] = blk
    return w


def _preprocess(x, edge_index, weight, bias):
    # self-loops are handled exactly on-chip (hself = dis^2 * h); only the
    # real edges go through the fp8 gather path
    msrc = np.asarray(edge_index[0], dtype=np.int64)
    mdst = np.asarray(edge_index[1], dtype=np.int64)
    loops = np.arange(N_NODES, dtype=np.int64)

    deg = np.bincount(np.concatenate([mdst, loops]),
                      minlength=N_NODES).astype(np.float32)
    dis = 1.0 / np.sqrt(deg)          # deg >= 1 because of self loops

    # source row in the gathered layout: half A -> c*LOC_A + r,
    # half B -> c*LOC_B + (r - LOC_A)
    sc = msrc // NLOC
    sr = msrc % NLOC
    half = (sr >= LOC_A).astype(np.int64)
    g = np.where(half == 0, sc * LOC_A + sr, sc * LOC_B + (sr - LOC_A))

    dc = mdst // NLOC                  # dst core
    dr = mdst % NLOC
    dt = dr // 128                     # dst tile within core
    dl = dr % 128                      # dst row within tile

    order = np.lexsort((g, half, dt, dc))
    g, dc, dt, dl, half = g[order], dc[order], dt[order], dl[order], half[order]

    key = (dc * T + dt) * 2 + half
    counts = np.bincount(key, minlength=C * T * 2).reshape(C, T, 2)
    blocks = -(-counts // 128)                       # ceil div
    B_A = blocks[:, :, 0].max(axis=0)                # [T]
    B_B = blocks[:, :, 1].max(axis=0)                # [T]

    idx_cols = int(8 * (B_A.sum() + B_B.sum()))
    blk_cols = int(B_A.sum() + B_B.sum())
    idx_np = np.zeros((C, 128, idx_cols), np.int16)
    oh_np = np.zeros((C, 128, blk_cols * 128), fp8)

    starts = np.zeros(C * T * 2 + 1, np.int64)
    np.cumsum(np.bincount(key, minlength=C * T * 2), out=starts[1:])

    dcol = np.arange(128)
    for c in range(C):
        icol = 0
        bcol = 0
        for t in range(T):
            for h, B in ((0, int(B_A[t])), (1, int(B_B[t]))):
                if B == 0:
                    continue
                k = (c * T + t) * 2 + h
                seg = slice(starts[k], starts[k + 1])
                n = starts[k + 1] - starts[k]
                cap = B * 128
                gi = np.zeros(cap, np.int64)
                gi[:n] = g[seg]
                dv = np.full(cap, -1.0, np.float32)
                dv[:n] = dl[seg]
                idx_np[c, :, icol:icol + 8 * B] = _wrap_idx(gi)
                ohb = (dv.reshape(B, 128)[:, :, None] == dcol[None, None, :])
                oh_np[c, :, bcol * 128:(bcol + B) * 128] = (
                    ohb.transpose(1, 0, 2).reshape(128, B * 128).astype(fp8))
                icol += 8 * B
                bcol += B

    w_bf = np.ascontiguousarray(weight.astype(bf16))
    # xT layout [128, KT, NPAD]: one DMA per GEMM chunk
    xT = np.zeros((C, 128, KT, NPAD), bf16)
    dis_np = np.zeros((C, 128, T), np.float32)
    for c in range(C):
        xs = x[c * NLOC:(c + 1) * NLOC]
        xt = np.zeros((D_IN, NPAD), bf16)
        xt[:, :NLOC] = xs.T.astype(bf16)
        xT[c] = xt.reshape(KT, 128, NPAD).transpose(1, 0, 2)
        dis_np[c, :, :] = np.pad(dis[c * NLOC:(c + 1) * NLOC],
                                 (0, NPAD - NLOC)).reshape(T, 128).T

    bias_full = np.tile(np.asarray(bias, np.float32)[None, :], (128, 1))

    return dict(
        B_A=B_A, B_B=B_B, idx=idx_np, oh=oh_np, w=w_bf, xT=xT,
        dis=dis_np, bias=np.ascontiguousarray(bias_full),
    )


def _build(B_A, B_B, idx_cols, blk_cols):
    nc = bacc.Bacc("TRN2", target_bir_lowering=False, debug=False,
                   num_devices=C, num_swdge_queues=4,
                   dynamic_dma_scratch_size=36864)

    xT_t = nc.dram_tensor("xT", [128, KT * NPAD], BF16, kind="ExternalInput")
    w_t = nc.dram_tensor("w", [D_IN, D_OUT], BF16, kind="ExternalInput")
    dis_t = nc.dram_tensor("dis", [128, T], F32, kind="ExternalInput")
    bias_t = nc.dram_tensor("biasf", [128, D_OUT], F32, kind="ExternalInput")
    idx_t = nc.dram_tensor("idx", [128, idx_cols], I16, kind="ExternalInput")
    oh_t = nc.dram_tensor("oh", [128, blk_cols * 128], FP8,
                          kind="ExternalInput")
    out_t = nc.dram_tensor("out", [NPAD, D_OUT], F16, kind="ExternalOutput")

    xT, w, dis, biasf, idx, oh, out = (
        t.ap() for t in (xT_t, w_t, dis_t, bias_t, idx_t, oh_t, out_t))

    # per-tile column offsets into idx / oh
    icolA = np.zeros(T, np.int64)
    icolB = np.zeros(T, np.int64)
    bcol0 = np.zeros(T, np.int64)
    ic = bc = 0
    for t in range(T):
        icolA[t] = ic
        ic += 8 * int(B_A[t])
        icolB[t] = ic
        ic += 8 * int(B_B[t])
        bcol0[t] = bc
        bc += int(B_A[t]) + int(B_B[t])

    BA_MAX = int(B_A.max())
    BB_MAX = int(B_B.max())

    with tile.TileContext(nc) as tc:
        with tc.tile_pool(name="const", bufs=1) as constp, \
             tc.tile_pool(name="xk", bufs=3) as xkp, \
             tc.tile_pool(name="hl", bufs=3) as hlp, \
             tc.tile_pool(name="gath", bufs=3) as gp, \
             tc.tile_pool(name="ohp", bufs=3) as ohp, \
             tc.tile_pool(name="hsp", bufs=3) as hsp, \
             tc.tile_pool(name="epi", bufs=2) as epip, \
             tc.tile_pool(name="zp", bufs=LCH + 2) as zp, \
             tc.tile_pool(name="psum", bufs=4, space="PSUM") as psp, \
             tc.tile_pool(name="dram", bufs=1, space="DRAM") as dramp:

            # resident constants
            w_sb = constp.tile([128, KT, D_OUT], BF16)
            for k in range(KT):
                nc.sync.dma_start(out=w_sb[:, k, :], in_=w[k * 128:(k + 1) * 128, :])
            dis_sb = constp.tile([128, T], F32)
            nc.sync.dma_start(out=dis_sb[:], in_=dis[:])
            bias_sb = constp.tile([128, D_OUT], F32)
            nc.sync.dma_start(out=bias_sb[:], in_=biasf[:])
            idx_sb = constp.tile([128, idx_cols], I16)
            nc.sync.dma_start(out=idx_sb[:], in_=idx[:])

            h_locA = dramp.tile([LOC_A, D_OUT], FP8)
            h_locB = dramp.tile([LOC_B, D_OUT], FP8)
            h_A = dramp.tile([ROWS_A, D_OUT], FP8, addr_space="Shared")
            h_B = dramp.tile([ROWS_B, D_OUT], FP8, addr_space="Shared")
            h_self = dramp.tile([NPAD, D_OUT], BF16)

            # fixed rotating gather slots + manual semaphores
            if PREP_MODE:
                gaA = [constp.tile([128, BA_MAX, D_OUT], FP8, name=f"gaA{s}")
                       for s in range(NBUF)]
                gaB = [constp.tile([128, BB_MAX, D_OUT], FP8, name=f"gaB{s}")
                       for s in range(NBUF)]
                dsemA = [nc.alloc_semaphore(f"dsemA{s}") for s in range(NBUF)]
                dsemB = [nc.alloc_semaphore(f"dsemB{s}") for s in range(NBUF)]
                freeA = [nc.alloc_semaphore(f"freeA{s}") for s in range(NBUF)]
                freeB = [nc.alloc_semaphore(f"freeB{s}") for s in range(NBUF)]
                for sm in (*dsemA, *dsemB, *freeA, *freeB):
                    nc.sync.sem_clear(sm)

            # ---- phase 1: h' = dis_src * (x @ W) in fp8, two allgathers;
            # hself = dis^2 * h stashed exactly (bf16) for the epilogue ----
            xTv = xT.rearrange("p (k n) -> p k n", k=KT)
            for s in range(SUP):
                for t in range(SW // 128):
                    gt0 = s * (SW // 128) + t
                    xk = xkp.tile([128, KT, 128], BF16, name="xk")
                    nc.sync.dma_start(
                        out=xk[:],
                        in_=xTv[:, :, gt0 * 128:(gt0 + 1) * 128])
                    ph = psp.tile([128, D_OUT], F32, name="ph")
                    for k in range(KT):
                        nc.tensor.matmul(
                            ph[:], xk[:, k, :],
                            w_sb[:, k, :], start=(k == 0), stop=(k == KT - 1))
                    hloc = hlp.tile([128, D_OUT], FP8, name="hloc")
                    gt = s * (SW // 128) + t
                    nc.vector.tensor_scalar(
                        hloc[:], ph[:], dis_sb[:, gt:gt + 1], None,
                        mybir.AluOpType.mult)
                    hs = hlp.tile([128, D_OUT], BF16, name="hs")
                    nc.vector.tensor_scalar(
                        hs[:], ph[:], dis_sb[:, gt:gt + 1],
                        dis_sb[:, gt:gt + 1], mybir.AluOpType.mult,
                        mybir.AluOpType.mult)
                    # fold the bias in here (phase-1 DVE has slack) so the
                    # 3b epilogue needs one add fewer
                    nc.vector.tensor_tensor(hs[:], hs[:], bias_sb[:],
                                            mybir.AluOpType.add)
                    nc.sync.dma_start(
                        out=h_self[gt * 128:(gt + 1) * 128, :], in_=hs[:])
                    r0 = gt * 128
                    if r0 < LOC_A:
                        nc.sync.dma_start(out=h_locA[r0:r0 + 128, :], in_=hloc[:])
                    else:
                        nc.sync.dma_start(
                            out=h_locB[r0 - LOC_A:r0 - LOC_A + 128, :], in_=hloc[:])
                if s == SUP_A - 1:
                    ccA = nc.gpsimd.collective_compute(
                        "AllGather", mybir.AluOpType.bypass,
                        replica_groups=[list(range(C))],
                        ins=[h_locA.opt()], outs=[h_A.opt()])
            ccB = nc.gpsimd.collective_compute(
                "AllGather", mybir.AluOpType.bypass,
                replica_groups=[list(range(C))],
                ins=[h_locB.opt()], outs=[h_B.opt()])
            # Desc-gen preps. Emitted AFTER both collectives (a prep before
            # its half's collective reads h_* as WAR against the collective
            # write and deadlocks). Only the first NBUF preps are hoisted:
            # a prep reusing slot s carries a WAW wait on the DMA of the
            # gather NBUF tiles earlier, so it must sit AFTER that gather's
            # trigger in Pool program order — hence one prep per trigger
            # in the phase loops below.
            def prep_A(t):
                ba = int(B_A[t])
                icol = int(icolA[t])
                nc.gpsimd.dma_gather(
                    out_ap=gaA[t % NBUF][:, :ba, :], in_ap=h_A[:],
                    idxs_ap=idx_sb[:, icol:icol + 8 * ba],
                    num_idxs=ba * 128, num_idxs_reg=ba * 128,
                    elem_size=D_OUT, prepare_only=True,
                    sem=dsemA[t % NBUF], queue_num=(t % 2) * 2)

            def prep_B(t):
                bb = int(B_B[t])
                icol = int(icolB[t])
                nc.gpsimd.dma_gather(
                    out_ap=gaB[t % NBUF][:, :bb, :], in_ap=h_B[:],
                    idxs_ap=idx_sb[:, icol:icol + 8 * bb],
                    num_idxs=bb * 128, num_idxs_reg=bb * 128,
                    elem_size=D_OUT, prepare_only=True,
                    sem=dsemB[t % NBUF], queue_num=(t % 2) * 2 + 1)

            if PREP_MODE:
                for t in range(NBUF):
                    prep_A(t)
                for t in range(NBUF):
                    prep_B(t)

            def agg_matmuls(acc, oh_sb, ga, nb, is_first, is_last):
                """Accumulate nb indicator blocks into acc via fp8 DoubleRow.
                Returns the last matmul instruction."""
                pairs = nb // 2
                odd = nb % 2
                mm = None
                for p in range(pairs):
                    mm = nc.tensor.matmul(
                        acc[:], oh_sb[:, 2 * p:2 * p + 2, :],
                        ga[:, 2 * p:2 * p + 2, :],
                        start=(is_first and p == 0),
                        stop=(is_last and odd == 0 and p == pairs - 1),
                        perf_mode=DR)
                if odd:
                    b = nb - 1
                    mm = nc.tensor.matmul(
                        acc[:], oh_sb[:, b, :], ga[:, b, :],
                        start=(is_first and pairs == 0), stop=is_last)
                return mm

            # ---- phase 3a: A-half gathers + matmuls, partials to SBUF ----
            partial = constp.tile([128, T, D_OUT], FP8)
            for t in range(T):
                ba = int(B_A[t])
                s = t % NBUF
                rnd = t // NBUF
                if PREP_MODE:
                    if rnd > 0:
                        nc.gpsimd.wait_ge(freeA[s], rnd)
                    trig = nc.gpsimd.trigger_dma(
                        count=1, queue_num=(t % 2) * 2)
                    if t < 2:
                        _add_dep_helper(trig.ins, ccA.ins, sync=True,
                                        reason="A gathers wait AllGather A")
                    if t + NBUF < T:
                        prep_A(t + NBUF)
                    ga_t = gaA[s]
                else:
                    ga_t = gp.tile([128, BA_MAX, D_OUT], FP8, name="ga",
                                   tag="ga")
                    icol = int(icolA[t])
                    nc.gpsimd.dma_gather(
                        out_ap=ga_t[:, :ba, :], in_ap=h_A[:],
                        idxs_ap=idx_sb[:, icol:icol + 8 * ba],
                        num_idxs=ba * 128, num_idxs_reg=ba * 128,
                        elem_size=D_OUT)
                oh_a = ohp.tile([128, BA_MAX, 128], FP8, name="oh_a", tag="oh_a")
                b0 = int(bcol0[t])
                nc.sync.dma_start(out=oh_a[:, :ba, :],
                                  in_=oh[:, b0 * 128:(b0 + ba) * 128])
                if PREP_MODE:
                    nc.tensor.wait_ge(dsemA[s], 16 * (rnd + 1))
                pa = psp.tile([128, D_OUT], F32, name="pa", tag="ph")
                agg_matmuls(pa, oh_a, ga_t, ba, True, True)
                # copy reads the finished PSUM group, so it retires after the
                # last matmul touching the gather slot; the nop behind it
                # (DVE is in-order) carries the slot-free signal
                nc.vector.tensor_copy(partial[:, t, :], pa[:])
                if PREP_MODE:
                    nc.vector.nop().then_inc(freeA[s], 1)

            # ---- phase 3b: B-half gathers + matmuls + epilogue ----
            smbuf = constp.tile([128, T], F32)
            lsebuf = constp.tile([128, T], F32)
            nlsebuf = constp.tile([128, T], F32)
            exscr = constp.tile([128, D_OUT], F32)
            ztiles = [None] * T

            def flush_ln(c0, c1):
                """Ln over tiles [c0, c1), then final subtract + store.
                The subtract runs on the Activation engine as
                Identity(z + (-lse)) — keeps it off the busier DVE."""
                nc.scalar.activation(lsebuf[:, c0:c1], smbuf[:, c0:c1],
                                     mybir.ActivationFunctionType.Ln)
                nc.vector.tensor_scalar(
                    nlsebuf[:, c0:c1], lsebuf[:, c0:c1], -1.0, None,
                    mybir.AluOpType.mult)
                for u in range(c0, c1):
                    res = epip.tile([128, D_OUT], F16, name="res")
                    nc.scalar.activation(
                        res[:], ztiles[u][:],
                        mybir.ActivationFunctionType.Identity,
                        bias=nlsebuf[:, u:u + 1])
                    nc.sync.dma_start(out=out[u * 128:(u + 1) * 128, :],
                                      in_=res[:])

            for t in range(T):
                bb = int(B_B[t])
                s = t % NBUF
                rnd = t // NBUF
                if PREP_MODE:
                    if rnd > 0:
                        nc.gpsimd.wait_ge(freeB[s], rnd)
                    trig = nc.gpsimd.trigger_dma(
                        count=1, queue_num=(t % 2) * 2 + 1)
                    if t < 2:
                        _add_dep_helper(trig.ins, ccB.ins, sync=True,
                                        reason="B gathers wait AllGather B")
                    if t + NBUF < T:
                        prep_B(t + NBUF)
                    gb_t = gaB[s]
                else:
                    gb_t = gp.tile([128, BB_MAX, D_OUT], FP8, name="gb",
                                   tag="gb")
                    icol = int(icolB[t])
                    nc.gpsimd.dma_gather(
                        out_ap=gb_t[:, :bb, :], in_ap=h_B[:],
                        idxs_ap=idx_sb[:, icol:icol + 8 * bb],
                        num_idxs=bb * 128, num_idxs_reg=bb * 128,
                        elem_size=D_OUT)
                oh_sb = ohp.tile([128, BB_MAX, 128], FP8, name="oh_b")
                b0 = int(bcol0[t]) + int(B_A[t])
                nc.sync.dma_start(out=oh_sb[:, :bb, :],
                                  in_=oh[:, b0 * 128:(b0 + bb) * 128])
                hst = hsp.tile([128, D_OUT], BF16, name="hst")
                nc.sync.dma_start(out=hst[:],
                                  in_=h_self[t * 128:(t + 1) * 128, :])
                if PREP_MODE:
                    nc.tensor.wait_ge(dsemB[s], 16 * (rnd + 1))
                acc = psp.tile([128, D_OUT], F32, name="acc")
                agg_matmuls(acc, oh_sb, gb_t, bb, True, True)

                # epilogue: z = dis_dst*(acc + partial) + hself + bias;
                # log_softmax without max-subtraction (|z| <~ 3).
                zsum = epip.tile([128, D_OUT], F32, name="zsum")
                nc.vector.tensor_tensor(zsum[:], acc[:], partial[:, t, :],
                                        mybir.AluOpType.add)
                if PREP_MODE:
                    nc.vector.nop().then_inc(freeB[s], 1)
                z = zp.tile([128, D_OUT], F32, name="z")
                ztiles[t] = z
                nc.scalar.activation(z[:], zsum[:],
                                     mybir.ActivationFunctionType.Identity,
                                     scale=dis_sb[:, t:t + 1])
                nc.vector.tensor_tensor(z[:], z[:], hst[:],
                                        mybir.AluOpType.add)
                nc.scalar.activation(exscr[:], z[:],
                                     mybir.ActivationFunctionType.Exp,
                                     accum_out=smbuf[:, t:t + 1])
                if t % LCH == LCH - 1:
                    flush_ln(t - LCH + 1, t + 1)
            if T % LCH:
                flush_ln(T - T % LCH, T)

    nc.compile()
    return nc


def kernel(x, edge_index, weight, bias):
    global LAST_RESULTS
    x = np.asarray(x, dtype=np.float32)
    weight = np.asarray(weight, dtype=np.float32)
    bias = np.asarray(bias, dtype=np.float32)

    pp = _preprocess(x, edge_index, weight, bias)
    idx_cols = pp["idx"].shape[2]
    blk_cols = pp["oh"].shape[2] // 128
    nc = _build(pp["B_A"], pp["B_B"], idx_cols, blk_cols)

    in_maps = []
    for c in range(C):
        in_maps.append({
            "xT": np.ascontiguousarray(pp["xT"][c]).reshape(128, KT * NPAD),
            "w": pp["w"],
            "dis": np.ascontiguousarray(pp["dis"][c]),
            "biasf": pp["bias"],
            "idx": np.ascontiguousarray(pp["idx"][c]),
            "oh": np.ascontiguousarray(pp["oh"][c]),
        })

    res = run_bass_kernel_spmd(nc, in_maps, core_ids=list(range(C)))
    LAST_RESULTS = res

    out = np.empty((N_NODES, D_OUT), np.float32)
    for c in range(C):
        out[c * NLOC:(c + 1) * NLOC] = (
            res.results[c]["out"][:NLOC].astype(np.float32))
    return out
